# revision 7
# baseline (speedup 1.0000x reference)
"""DirGCNConv on 8 Trainium2 NeuronCores.

Math (reference):
  A = dense 0/1 adjacency from edge_index (coalesced), At = A.T
  SO_in  = mask(At@A),  SO_out = mask(A@At)   (mask: zero where edge / diagonal)
  y = 0.35*h1 + 0.35*h2 + 0.15*h3 + 0.15*h4,  h = dir_norm(M) @ x @ W.T + b

Sharding: each core c owns output rows Rc = [512c, 512c+512).
Everything on device is kept in a transposed "C layout" [K on partitions,
row-block m on free] so that matmul lhsT operands never need transposing:
  C_in  = (At@A)[:, Rc] = SO_in[Rc, :].T  (pre-mask symmetry)
  C_out = (A@At)[:, Rc] = SO_out[Rc, :].T
Masks come for free: the mask of C_in is (acol == 0) offdiag where acol =
A[:, Rc] is already resident as the phase-1 rhs. Second-order counts and 0/1
adjacencies are exact in bf16, so phase 1 runs at full bf16 PE speed. SpMMs
run with x split into bf16 hi+lo halves (exact to ~2^-18) against the exact
bf16 matrices. Per-node dir_norm scales are folded outside the matmuls.
"""
import numpy as np
import ml_dtypes
from contextlib import ExitStack

N = 4096
P = 128
KC = N // P          # 32 k-chunks
B = 512              # rows per core
MC = B // P          # 4 row chunks per core
D = 256
DH = D // P          # 2 feature chunks
NCORES = 8
SENT = 1.0e9         # "no diagonal in this chunk" sentinel

_CACHE = {}


def _build_nc():
    import concourse.bacc as bacc
    import concourse.mybir as mybir
    import concourse.tile as tile
    from concourse.alu_op_type import AluOpType as op
    import bass_rust
    AF = bass_rust.ActivationFunctionType
    f32 = mybir.dt.float32
    bf16 = mybir.dt.bfloat16
    i32 = mybir.dt.int32

    nc = bacc.Bacc("TRN2", num_devices=NCORES)

    a_strips = nc.dram_tensor("a_strips", [KC, P, KC, P], bf16, kind="ExternalInput")
    at_strips = nc.dram_tensor("at_strips", [KC, P, KC, P], bf16, kind="ExternalInput")
    acol_d = nc.dram_tensor("acol", [N, B], bf16, kind="ExternalInput")
    atcol_d = nc.dram_tensor("atcol", [N, B], bf16, kind="ExternalInput")
    x_d = nc.dram_tensor("xin", [N, D], f32, kind="ExternalInput")
    ia_s2d_d = nc.dram_tensor("ia_s2d", [P, KC], f32, kind="ExternalInput")
    ia_d2s_d = nc.dram_tensor("ia_d2s", [P, KC], f32, kind="ExternalInput")
    oa_s2d_d = nc.dram_tensor("oa_s2d", [P, MC], f32, kind="ExternalInput")
    oa_d2s_d = nc.dram_tensor("oa_d2s", [P, MC], f32, kind="ExternalInput")
    wsrcT_d = nc.dram_tensor("wsrcT", [D, D], f32, kind="ExternalInput")
    wdstT_d = nc.dram_tensor("wdstT", [D, D], f32, kind="ExternalInput")
    dm0_d = nc.dram_tensor("dm0", [P, KC], f32, kind="ExternalInput")
    y_d = nc.dram_tensor("y", [B, D], f32, kind="ExternalOutput")

    with tile.TileContext(nc) as tc:
        with ExitStack() as ctx:
            cpool = ctx.enter_context(tc.tile_pool(name="const", bufs=1))
            strips = ctx.enter_context(tc.tile_pool(name="strips", bufs=2))
            xw = ctx.enter_context(tc.tile_pool(name="xw", bufs=2))
            evp = ctx.enter_context(tc.tile_pool(name="evp", bufs=3))
            tiny = ctx.enter_context(tc.tile_pool(name="tiny", bufs=1))
            gevp = ctx.enter_context(tc.tile_pool(name="gevp", bufs=2))
            ps_fo = ctx.enter_context(tc.tile_pool(name="ps_fo", bufs=4, space="PSUM"))
            ps_c = ctx.enter_context(tc.tile_pool(name="ps_c", bufs=2, space="PSUM"))
            ps_rs = ctx.enter_context(tc.tile_pool(name="ps_rs", bufs=1, space="PSUM"))
            dram = ctx.enter_context(tc.tile_pool(name="dram", bufs=1, space="DRAM"))

            # ---- resident constants ----
            acol_sb = cpool.tile([P, KC, B], bf16)
            nc.sync.dma_start(out=acol_sb[:], in_=acol_d.rearrange("(kc p) j -> p kc j", p=P))
            atcol_sb = cpool.tile([P, KC, B], bf16)
            nc.sync.dma_start(out=atcol_sb[:], in_=atcol_d.rearrange("(kc p) j -> p kc j", p=P))
            ia_sb = {}
            for name, d in (("s2d", ia_s2d_d), ("d2s", ia_d2s_d)):
                t = cpool.tile([P, KC], f32, tag=f"ia_{name}", name=f"ia_{name}")
                nc.sync.dma_start(out=t[:], in_=d[:])
                ia_sb[name] = t
            oa_sb = {}
            for name, d in (("s2d", oa_s2d_d), ("d2s", oa_d2s_d)):
                t = cpool.tile([P, MC], f32, tag=f"oa_{name}", name=f"oa_{name}")
                nc.sync.dma_start(out=t[:], in_=d[:])
                oa_sb[name] = t
            w_sb = {}
            for name, d in (("src", wsrcT_d), ("dst", wdstT_d)):
                t = cpool.tile([P, DH, D], f32, tag=f"w_{name}", name=f"w_{name}")
                nc.sync.dma_start(out=t[:], in_=d.rearrange("(kc p) j -> p kc j", p=P))
                w_sb[name] = t
            dm0_sb = cpool.tile([P, KC], f32)
            nc.sync.dma_start(out=dm0_sb[:], in_=dm0_d[:])

            idxi = tiny.tile([P, B], i32)
            nc.gpsimd.iota(idxi[:], pattern=[[1, B]], base=0, channel_multiplier=-1)
            idxf = cpool.tile([P, B], f32)
            nc.vector.tensor_copy(out=idxf[:], in_=idxi[:])
            ident = cpool.tile([P, P], f32)
            nc.vector.tensor_scalar(out=ident[:], in0=idxf[:, :P], scalar1=0.0,
                                    scalar2=None, op0=op.is_equal)
            ones_col = cpool.tile([P, 1], bf16)
            nc.vector.memset(ones_col[:], 1.0)

            mc_sb = {"in": cpool.tile([P, KC, B], bf16, tag="mcin", name="mcin"),
                     "out": cpool.tile([P, KC, B], bf16, tag="mcout", name="mcout")}
            # aggT tiles: term -> [P, DH, B] f32
            TERMS = ("fo_s2d", "fo_d2s", "so_out", "so_in")
            aggT = {t: cpool.tile([P, DH, B], f32, tag=f"agg_{t}", name=f"agg_{t}") for t in TERMS}
            colp = {s: cpool.tile([P, KC], f32, tag=f"colp_{s}", name=f"colp_{s}") for s in ("in", "out")}
            iso_sb = {s: cpool.tile([P, KC], f32, tag=f"iso_{s}", name=f"iso_{s}") for s in ("in", "out")}
            oso_sb = {s: cpool.tile([P, MC], f32, tag=f"oso_{s}", name=f"oso_{s}") for s in ("in", "out")}
            ysb = cpool.tile([P, MC, D], f32)

            ccin = dram.tile([2, N], f32)
            ccout = dram.tile([2, N], f32)
            oso_dram = dram.tile([2, B], f32)

            def split_x(xs):
                """f32 [P, D] -> (hi, lo) bf16 tiles, hi+lo == xs to ~2^-18."""
                xhi = xw.tile([P, D], bf16, tag="xhi", name="xhi")
                nc.vector.tensor_copy(out=xhi[:], in_=xs[:])
                xhif = xw.tile([P, D], f32, tag="xhif", name="xhif")
                nc.vector.tensor_copy(out=xhif[:], in_=xhi[:])
                xlo = xw.tile([P, D], bf16, tag="xlo", name="xlo")
                nc.vector.tensor_tensor(out=xlo[:], in0=xs[:], in1=xhif[:], op=op.subtract)
                return xhi, xlo

            # ============ P0: first-order SpMMs (PE warm-up work) ============
            fo_ps = {(t, dh): ps_fo.tile([P, B], f32, tag="fo", name=f"fo_{t}_{dh}")
                     for t in ("s2d", "d2s") for dh in range(DH)}
            fo_rhs = {"s2d": atcol_sb, "d2s": acol_sb}
            for k in range(KC):
                xk = xw.tile([P, D], f32, tag="xk", name="xk")
                nc.sync.dma_start(out=xk[:], in_=x_d[k * P:(k + 1) * P, :])
                for t in ("s2d", "d2s"):
                    xs = xw.tile([P, D], f32, tag="xs", name="xs")
                    nc.vector.tensor_scalar(out=xs[:], in0=xk[:], scalar1=ia_sb[t][:, k:k + 1],
                                            scalar2=None, op0=op.mult)
                    xhi, xlo = split_x(xs)
                    rhs = fo_rhs[t][:, k, :]
                    for dh in range(DH):
                        for half, lhsT in ((0, xhi), (1, xlo)):
                            nc.tensor.matmul(fo_ps[(t, dh)][:],
                                             lhsT=lhsT[:, dh * P:(dh + 1) * P], rhs=rhs,
                                             start=(k == 0 and half == 0),
                                             stop=(k == KC - 1 and half == 1))
            for t, name in (("s2d", "fo_s2d"), ("d2s", "fo_d2s")):
                for dh in range(DH):
                    nc.vector.tensor_copy(out=aggT[name][:, dh, :], in_=fo_ps[(t, dh)][:])

            # ============ P1: second-order C blocks, mask, degree sums ============
            def phase1(side, strips_d, colsb):
                """side 'in': C_in = (At@A)[:,Rc]: lhsT = A strips, rhs/mask = acol.
                   side 'out': C_out = (A@At)[:,Rc]: lhsT = At strips, rhs/mask = atcol."""
                mc = mc_sb[side]
                for i in range(KC):
                    strip = strips.tile([P, KC * P], bf16, tag="strip", name="strip")
                    nc.sync.dma_start(out=strip[:], in_=strips_d[i].rearrange("p a b -> p (a b)"))
                    cps = ps_c.tile([P, B], f32, tag="c", name="c")
                    for nbr in range(KC):
                        nc.tensor.matmul(cps[:], lhsT=strip[:, nbr * P:(nbr + 1) * P],
                                         rhs=colsb[:, nbr, :],
                                         start=(nbr == 0), stop=(nbr == KC - 1))
                    cbf = evp.tile([P, B], bf16, tag="cbf", name="cbf")
                    nc.vector.tensor_copy(out=cbf[:], in_=cps[:])
                    # zero where edge exists (mask tile == the resident column block)
                    nc.vector.scalar_tensor_tensor(out=mc[:, i, :], in0=colsb[:, i, :],
                                                   scalar=0.0, in1=cbf[:],
                                                   op0=op.is_equal, op1=op.mult)
                    # zero the diagonal (data-driven: dm0 = m0 for diag chunks, else 1e9)
                    dm = evp.tile([P, B], bf16, tag="dm", name="dm")
                    nc.vector.tensor_scalar(out=dm[:], in0=idxf[:], scalar1=dm0_sb[:, i:i + 1],
                                            scalar2=None, op0=op.not_equal)
                    nc.vector.tensor_tensor(out=mc[:, i, :], in0=mc[:, i, :], in1=dm[:],
                                            op=op.mult)
                    # partial column sums (free-dim reduce)
                    nc.vector.reduce_sum(colp[side][:, i:i + 1], mc[:, i, :],
                                         axis=bass_rust.AxisListType.X)
                # row sums via ones-matmul over the masked tiles
                rs = ps_rs.tile([1, B], f32, tag="rs", name="rs")
                for i in range(KC):
                    nc.tensor.matmul(rs[:], lhsT=ones_col[:], rhs=mc[:, i, :],
                                     start=(i == 0), stop=(i == KC - 1))
                # o_so = 0.15 * rsqrt(rowsum) * (rowsum > 0), via sqrt+recip
                ind = tiny.tile([1, B], f32, tag="rind", name="rind")
                nc.vector.tensor_scalar(out=ind[:], in0=rs[:], scalar1=0.0,
                                        scalar2=None, op0=op.is_gt)
                val = tiny.tile([1, B], f32, tag="rval", name="rval")
                nc.vector.tensor_scalar(out=val[:], in0=rs[:], scalar1=1e-30,
                                        scalar2=None, op0=op.max)
                nc.scalar.activation(out=val[:], in_=val[:], func=AF.Sqrt,
                                     scale=1.0 / (0.15 * 0.15))
                nc.vector.reciprocal(out=val[:], in_=val[:])
                nc.vector.tensor_tensor(out=val[:], in0=val[:], in1=ind[:], op=op.mult)
                si = 0 if side == "in" else 1
                nc.sync.dma_start(out=oso_dram[si], in_=val[:])
                nc.sync.dma_start(out=oso_sb[side][:],
                                  in_=oso_dram[si].rearrange("(mc p) -> p mc", p=P))
                # ship partial colsums to the collective input
                nc.sync.dma_start(out=ccin[si].rearrange("(kc p) -> p kc", p=P),
                                  in_=colp[side][:])

            phase1("in", a_strips, acol_sb)
            phase1("out", at_strips, atcol_sb)

            # ============ P2: AllReduce column sums -> i_so scales ============
            nc.gpsimd.collective_compute(
                "AllReduce", mybir.AluOpType.add,
                replica_groups=[list(range(NCORES))],
                ins=[ccin.opt()], outs=[ccout.opt()])
            for si, side in ((0, "in"), (1, "out")):
                raw = tiny.tile([P, KC], f32, tag="israw", name="israw")
                nc.sync.dma_start(out=raw[:], in_=ccout[si].rearrange("(kc p) -> p kc", p=P))
                ind = tiny.tile([P, KC], f32, tag="isind", name="isind")
                nc.vector.tensor_scalar(out=ind[:], in0=raw[:], scalar1=0.0,
                                        scalar2=None, op0=op.is_gt)
                nc.vector.tensor_scalar(out=raw[:], in0=raw[:], scalar1=1e-30,
                                        scalar2=None, op0=op.max)
                nc.scalar.activation(out=raw[:], in_=raw[:], func=AF.Sqrt,
                                     scale=1.0)
                nc.vector.reciprocal(out=raw[:], in_=raw[:])
                nc.vector.tensor_tensor(out=iso_sb[side][:], in0=raw[:], in1=ind[:],
                                        op=op.mult)

            # ============ P3: second-order SpMMs ============
            so_ps = {(sd, dh): ps_fo.tile([P, B], f32, tag="fo", name=f"so_{sd}_{dh}")
                     for sd in ("out", "in") for dh in range(DH)}
            for k in range(KC):
                xk = xw.tile([P, D], f32, tag="xk", name="xk")
                nc.sync.dma_start(out=xk[:], in_=x_d[k * P:(k + 1) * P, :])
                for side in ("out", "in"):
                    xs = xw.tile([P, D], f32, tag="xs", name="xs")
                    nc.vector.tensor_scalar(out=xs[:], in0=xk[:],
                                            scalar1=iso_sb[side][:, k:k + 1],
                                            scalar2=None, op0=op.mult)
                    xhi, xlo = split_x(xs)
                    rhs = mc_sb[side][:, k, :]
                    for dh in range(DH):
                        for half, lhsT in ((0, xhi), (1, xlo)):
                            nc.tensor.matmul(so_ps[(side, dh)][:],
                                             lhsT=lhsT[:, dh * P:(dh + 1) * P], rhs=rhs,
                                             start=(k == 0 and half == 0),
                                             stop=(k == KC - 1 and half == 1))
            for side, name in (("out", "so_out"), ("in", "so_in")):
                for dh in range(DH):
                    nc.vector.tensor_copy(out=aggT[name][:, dh, :], in_=so_ps[(side, dh)][:])

            # ============ P4+P5: W GEMMs (fp32), transpose, o-scaled accumulate ====
            TW = {"fo_s2d": "src", "fo_d2s": "dst", "so_out": "src", "so_in": "dst"}
            TO = {"fo_s2d": oa_sb["s2d"], "fo_d2s": oa_sb["d2s"],
                  "so_out": oso_sb["out"], "so_in": oso_sb["in"]}
            for ti, term in enumerate(TERMS):
                w = w_sb[TW[term]]
                ot = TO[term]
                for dh in range(DH):
                    g = ps_c.tile([P, B], f32, tag="c", name="g")
                    for kh in range(DH):
                        nc.tensor.matmul(g[:], lhsT=w[:, kh, dh * P:(dh + 1) * P],
                                         rhs=aggT[term][:, kh, :],
                                         start=(kh == 0), stop=(kh == DH - 1))
                    gev = gevp.tile([P, B], f32, tag="gev", name="gev")
                    nc.vector.tensor_copy(out=gev[:], in_=g[:])
                    for mh in range(MC):
                        tp = ps_fo.tile([P, P], f32, tag="fo", name="tp")
                        nc.tensor.transpose(tp[:], gev[:, mh * P:(mh + 1) * P], ident[:])
                        dst = ysb[:, mh, dh * P:(dh + 1) * P]
                        if ti == 0:
                            nc.vector.tensor_scalar(out=dst, in0=tp[:],
                                                    scalar1=ot[:, mh:mh + 1],
                                                    scalar2=None, op0=op.mult)
                        else:
                            nc.vector.scalar_tensor_tensor(out=dst, in0=tp[:],
                                                           scalar=ot[:, mh:mh + 1],
                                                           in1=dst, op0=op.mult,
                                                           op1=op.add)

            nc.sync.dma_start(out=y_d.rearrange("(mc p) d -> p mc d", p=P), in_=ysb[:])

    nc.finalize()
    return nc


def _host_prep(x, edge_index):
    bf16 = ml_dtypes.bfloat16
    ei = np.asarray(edge_index).astype(np.int64)
    lin = ei[0] * N + ei[1]
    uniq = np.unique(lin)
    A = np.zeros(N * N, np.float32)
    A[uniq] = 1.0
    A = A.reshape(N, N)
    dr = np.bincount((uniq // N).astype(np.int64), minlength=N).astype(np.float64)
    dc = np.bincount((uniq % N).astype(np.int64), minlength=N).astype(np.float64)

    def rnorm(d):
        return np.where(d > 0, 1.0 / np.sqrt(np.maximum(d, 1e-30)), 0.0).astype(np.float32)

    rdr, rdc = rnorm(dr), rnorm(dc)
    Abf = A.astype(bf16)
    Atbf = np.ascontiguousarray(Abf.T)
    a_strips = np.ascontiguousarray(Abf.reshape(KC, P, KC, P).transpose(2, 1, 0, 3))
    at_strips = np.ascontiguousarray(Atbf.reshape(KC, P, KC, P).transpose(2, 1, 0, 3))
    return Abf, Atbf, a_strips, at_strips, rdr, rdc


def kernel(x, edge_index, W_src, b_src, W_dst, b_dst):
    from concourse.bass_utils import run_bass_kernel_spmd

    x = np.asarray(x, dtype=np.float32)
    W_src = np.asarray(W_src, dtype=np.float32)
    W_dst = np.asarray(W_dst, dtype=np.float32)
    b_src = np.asarray(b_src, dtype=np.float32)
    b_dst = np.asarray(b_dst, dtype=np.float32)

    Abf, Atbf, a_strips, at_strips, rdr, rdc = _host_prep(x, edge_index)

    ia_s2d = np.ascontiguousarray(rdc.reshape(KC, P).T)   # [P, KC]
    ia_d2s = np.ascontiguousarray(rdr.reshape(KC, P).T)
    wsrcT = np.ascontiguousarray(W_src.T)
    wdstT = np.ascontiguousarray(W_dst.T)

    in_maps = []
    for c in range(NCORES):
        sl = slice(c * B, (c + 1) * B)
        dm0 = np.full((P, KC), SENT, np.float32)
        for i in range(c * MC, c * MC + MC):
            dm0[:, i] = np.float32(i * P - c * B)
        in_maps.append({
            "a_strips": a_strips, "at_strips": at_strips,
            "acol": np.ascontiguousarray(Abf[:, sl]),
            "atcol": np.ascontiguousarray(Atbf[:, sl]),
            "xin": x,
            "ia_s2d": ia_s2d, "ia_d2s": ia_d2s,
            "oa_s2d": np.ascontiguousarray((0.35 * rdr[sl]).reshape(MC, P).T),
            "oa_d2s": np.ascontiguousarray((0.35 * rdc[sl]).reshape(MC, P).T),
            "wsrcT": wsrcT, "wdstT": wdstT,
            "dm0": dm0,
        })

    if "nc" not in _CACHE:
        _CACHE["nc"] = _build_nc()
    res = run_bass_kernel_spmd(_CACHE["nc"], in_maps, list(range(NCORES)))
    y = np.concatenate([res.results[c]["y"] for c in range(NCORES)], axis=0)
    y = y + 0.5 * (b_src + b_dst)[None, :]
    return np.ascontiguousarray(y.astype(np.float32))
